# revision 1
# baseline (speedup 1.0000x reference)
"""CenterLoss kernel for Trainium2 (8 NeuronCores, data-parallel over batch).

loss = mean_i( clip( ||x_i - centers[labels[i]]||^2, 1e-12, 1e12 ) )

Instead of materializing the [B, C] distance matrix (as the reference does),
we gather the labeled center row per sample with an indirect DMA and compute
the squared distance directly: O(B*D) work instead of O(B*C*D).

Sharding: x/labels split into 8 batch shards of 1024 rows; centers replicated.
Each core emits its shard's partial sum of clipped distances; the host sums
the 8 partials and divides by the global batch (the sanctioned all-reduce).
"""

import sys

import numpy as np

if "/opt/trn_rl_repo" not in sys.path:
    sys.path.insert(0, "/opt/trn_rl_repo")

_B, _D, _C = 8192, 512, 8000
_N_CORES = 8
_B_LOC = _B // _N_CORES  # 1024 rows per core
_P = 128
_N_CHUNK = _B_LOC // _P  # 8 chunks of 128 rows
_CLAMP_MIN, _CLAMP_MAX = 1e-12, 1e12

_cache: dict = {}


def _build():
    import concourse.bass as bass
    import concourse.tile as tile
    from concourse import bacc, mybir

    nc = bacc.Bacc(
        "TRN2",
        debug=False,
        enable_asserts=False,
        target_bir_lowering=False,
        num_devices=_N_CORES,
    )
    x_d = nc.dram_tensor("x", [_B_LOC, _D], mybir.dt.float32, kind="ExternalInput")
    lab_d = nc.dram_tensor("labels", [_B_LOC], mybir.dt.int32, kind="ExternalInput")
    cen_d = nc.dram_tensor("centers", [_C, _D], mybir.dt.float32, kind="ExternalInput")
    out_d = nc.dram_tensor("out", [1, 1], mybir.dt.float32, kind="ExternalOutput")

    with tile.TileContext(nc) as tc:
        with (
            tc.tile_pool(name="big", bufs=1) as big,
            tc.tile_pool(name="work", bufs=3) as work,
            tc.tile_pool(name="misc", bufs=1) as misc,
            tc.tile_pool(name="psum", bufs=1, space="PSUM") as psum,
        ):
            # Sample s = p*8 + n lives at (partition p, chunk n): each SBUF
            # partition reads 8 contiguous rows (16 KB) -> line-rate DMA.
            idx = misc.tile([_P, _N_CHUNK], mybir.dt.int32)
            nc.sync.dma_start(
                out=idx[:], in_=lab_d.ap().rearrange("(p n) -> p n", p=_P)
            )
            xsb = big.tile([_P, _N_CHUNK * _D], mybir.dt.float32)
            nc.sync.dma_start(
                out=xsb[:], in_=x_d.ap().rearrange("(p n) d -> p (n d)", p=_P)
            )

            ones = misc.tile([_P, 1], mybir.dt.float32)
            nc.vector.memset(ones[:], 1.0)

            dist = misc.tile([_P, _N_CHUNK], mybir.dt.float32)

            for n in range(_N_CHUNK):
                g = work.tile([_P, _D], mybir.dt.float32, tag="g")
                nc.gpsimd.indirect_dma_start(
                    out=g[:],
                    out_offset=None,
                    in_=cen_d.ap(),
                    in_offset=bass.IndirectOffsetOnAxis(ap=idx[:, n : n + 1], axis=0),
                )
                diff = work.tile([_P, _D], mybir.dt.float32, tag="diff")
                nc.vector.tensor_tensor(
                    out=diff[:],
                    in0=xsb[:, n * _D : (n + 1) * _D],
                    in1=g[:],
                    op=mybir.AluOpType.subtract,
                )
                sq = work.tile([_P, _D], mybir.dt.float32, tag="sq")
                nc.scalar.activation(
                    out=sq[:],
                    in_=diff[:],
                    func=mybir.ActivationFunctionType.Square,
                    accum_out=dist[:, n : n + 1],
                )

            nc.vector.tensor_scalar_max(dist[:], dist[:], _CLAMP_MIN)
            nc.vector.tensor_scalar_min(dist[:], dist[:], _CLAMP_MAX)

            # Partition reduction: ones[128,1].T @ dist[128,8] -> psum[1,8].
            ps = psum.tile([1, _N_CHUNK], mybir.dt.float32, space="PSUM")
            nc.tensor.matmul(out=ps[:], lhsT=ones[:], rhs=dist[:], start=True, stop=True)
            res = misc.tile([1, 1], mybir.dt.float32)
            nc.vector.tensor_reduce(
                out=res[:], in_=ps[:], axis=mybir.AxisListType.X, op=mybir.AluOpType.add
            )
            nc.sync.dma_start(out=out_d.ap()[:, :], in_=res[:])
    nc.compile()
    return nc


def _run(x, labels, centers, trace=False, **hw_kwargs):
    from concourse import bass_utils

    if "nc" not in _cache:
        _cache["nc"] = _build()
    nc = _cache["nc"]

    x = np.ascontiguousarray(np.asarray(x, dtype=np.float32))
    labels = np.ascontiguousarray(np.asarray(labels).astype(np.int32))
    centers = np.ascontiguousarray(np.asarray(centers, dtype=np.float32))
    assert x.shape == (_B, _D) and labels.shape == (_B,) and centers.shape == (_C, _D)

    in_maps = []
    for c in range(_N_CORES):
        sl = slice(c * _B_LOC, (c + 1) * _B_LOC)
        in_maps.append({"x": x[sl], "labels": labels[sl], "centers": centers})

    r = bass_utils.run_bass_kernel_spmd(
        nc, in_maps, core_ids=list(range(_N_CORES)), trace=trace, **hw_kwargs
    )
    total = sum(float(res["out"][0, 0]) for res in r.results)
    return np.float32(total / _B), r


def kernel(x, labels, centers):
    out, _ = _run(x, labels, centers, trace=False)
    return out
